# revision 1
# baseline (speedup 1.0000x reference)
"""Trainium2 Bass kernel for the histogram-binning CTC-style loss.

reference:
    x: [T=512, B=32, V=10000] f32, label: [B=32, L=64] int
    counts[b,v] = histogram of non-blank labels; counts[b,0] = T - len_b
    loss = -sum_{b,v} (counts[b,v]/T) * log(mean_t x[t,b,v] + 1e-10) / B

Strategy (8 NeuronCores, data-parallel over batch, sparse gather):
    Only columns v with counts[b,v] != 0 contribute: <=64 unique labels +
    the blank per batch -> <=65 columns x 4 local batches = 260 columns
    per core out of 40000.  Each core receives a per-core COLUMN TABLE
    (int32 data) and WEIGHTS; the device reads each table entry into a
    sequencer register and issues a dynamic-offset DMA for that column
    ([128 t-partitions x 4 t-chunks x 1col]), round-robined over the three
    DMA-generation rings (sync/scalar HWDGE + gpsimd SWDGE).  The T-sum is
    a ones-vector matmul accumulated over the 4 t-chunks in PSUM, then
    ScalarE Ln (scale=1/T, bias=1e-10), VectorE dot with the weights, and
    a single partial scalar out per core; the host sums the 8 partials.

    One program serves any input: column tables are data, not code.
"""

import numpy as np

import concourse.bass as bass
import concourse.bacc as bacc
import concourse.mybir as mybir
import concourse.tile as tile
from concourse.bass_utils import run_bass_kernel_spmd

T = 512
B = 32
V = 10000
L = 64
NCORES = 8
BL = B // NCORES          # local batches per core
F = BL * V                # flattened (b, v) columns per core
TCH = T // 128            # T chunks of 128 partitions
PER = 65                  # column slots per batch (<=64 labels + blank)
NCOLS = BL * PER          # 260 gathered columns per core

_NC_CACHE = {}


def _build_nc(rings=("sync", "scalar", "gpsimd"), interleave=2, loop=1):
    n = NCOLS
    nc = bacc.Bacc()
    x_d = nc.declare_dram_parameter("x", [T, F], mybir.dt.float32, isOutput=False)
    c_d = nc.declare_dram_parameter("cols", [1, n], mybir.dt.int32, isOutput=False)
    w_d = nc.declare_dram_parameter("w", [1, n], mybir.dt.float32, isOutput=False)
    out_d = nc.declare_dram_parameter("out", [1, 1], mybir.dt.float32, isOutput=True)
    xr = x_d[:, :].rearrange("(c p) f -> p c f", p=128)  # [128, TCH, F]

    import contextlib
    with tile.TileContext(nc) as tc:
        loop_cm = tc.For_i(0, loop, 1) if loop > 1 else contextlib.nullcontext()
        with (
            loop_cm,
            tc.tile_pool(name="gp", bufs=1) as gp,
            tc.tile_pool(name="sp", bufs=1) as sp,
            tc.tile_pool(name="cp", bufs=1) as cp,
            tc.tile_pool(name="psum", bufs=1, space="PSUM") as psum,
        ):
            ones = cp.tile([128, 1], mybir.dt.bfloat16)
            nc.gpsimd.memset(ones[:], 1.0)
            biasv = cp.tile([1, 1], mybir.dt.float32)
            nc.gpsimd.memset(biasv[:], 1e-10)

            ct = cp.tile([1, n], mybir.dt.int32)
            nc.sync.dma_start(out=ct[:], in_=c_d[:, :])
            wt = sp.tile([1, n], mybir.dt.float32)
            nc.sync.dma_start(out=wt[:], in_=w_d[:, :])

            engs = [{"sync": nc.sync, "scalar": nc.scalar, "gpsimd": nc.gpsimd}[r]
                    for r in rings]
            ne = len(engs)
            xg = gp.tile([128, TCH, n], mybir.dt.float32)
            slots = [[] for _ in range(ne)]
            for i in range(n):
                slots[i % ne].append(i)
            for e, eng in enumerate(engs):
                my = slots[e]
                regs = [contextlib.ExitStack() for _ in range(interleave)]
                rhandles = []
                for k in range(interleave):
                    r = regs[k].enter_context(eng.register(f"col_e{e}_{k}"))
                    rhandles.append(r)
                for base in range(0, len(my), interleave):
                    grp = my[base:base + interleave]
                    offs = []
                    for k, i in enumerate(grp):
                        eng.reg_load(rhandles[k], ct[0:1, i:i + 1])
                        offs.append(eng.snap(rhandles[k]))
                    for k, i in enumerate(grp):
                        eng.dma_start(out=xg[:, :, i:i + 1],
                                      in_=xr[:, :, bass.ds(offs[k], 1)])
                for k in range(interleave):
                    regs[k].close()

            xgb = sp.tile([128, TCH, n], mybir.dt.bfloat16)
            nc.vector.tensor_copy(xgb[:], xg[:])
            ps = psum.tile([1, n], mybir.dt.float32)
            for c in range(TCH):
                nc.tensor.matmul(
                    ps[:], ones[:], xgb[:, c, :],
                    start=(c == 0), stop=(c == TCH - 1),
                )
            logv = sp.tile([1, n], mybir.dt.float32)
            nc.scalar.activation(
                logv[:], ps[:], mybir.ActivationFunctionType.Ln,
                bias=biasv[:], scale=1.0 / T,
            )
            prod = sp.tile([1, n], mybir.dt.float32)
            nc.vector.tensor_tensor(
                out=prod[:], in0=logv[:], in1=wt[:], op=mybir.AluOpType.mult,
            )
            total = cp.tile([1, 1], mybir.dt.float32)
            nc.vector.tensor_reduce(
                out=total[:], in_=prod[:], axis=mybir.AxisListType.X,
                op=mybir.AluOpType.add,
            )
            nc.sync.dma_start(out=out_d[:, :], in_=total[:])

    nc.finalize()
    return nc


def get_nc():
    if "nc" not in _NC_CACHE:
        _NC_CACHE["nc"] = _build_nc()
    return _NC_CACHE["nc"]


def plan_core(label_rows):
    """label_rows: [BL, L] labels for one core's batches.
    Returns cols [NCOLS] int32 flattened (b*V + v) indices and w [1, NCOLS]
    f32 count weights (blank slot weight = T - len_b; padding weight 0)."""
    cols = np.zeros(NCOLS, dtype=np.int32)
    w = np.zeros((1, NCOLS), dtype=np.float32)
    for b in range(BL):
        lab = np.asarray(label_rows[b])
        m = lab != 0
        vals, cnts = np.unique(lab[m], return_counts=True)
        assert len(vals) <= PER - 1
        base = b * PER
        cols[base] = b * V + 0
        w[0, base] = T - m.sum()
        cols[base + 1:base + 1 + len(vals)] = b * V + vals
        w[0, base + 1:base + 1 + len(vals)] = cnts
        cols[base + 1 + len(vals):base + PER] = b * V  # pad: col 0, weight 0
    return cols, w


def make_in_maps(x, label):
    x = np.ascontiguousarray(np.asarray(x, dtype=np.float32))
    label = np.asarray(label)
    in_maps = []
    for c in range(NCORES):
        xs = np.ascontiguousarray(x[:, c * BL:(c + 1) * BL, :]).reshape(T, F)
        cols, w = plan_core(label[c * BL:(c + 1) * BL])
        in_maps.append({"x": xs, "cols": cols.reshape(1, -1), "w": w})
    return in_maps


def kernel(x, label):
    nc = get_nc()
    in_maps = make_in_maps(x, label)
    res = run_bass_kernel_spmd(nc, in_maps, core_ids=list(range(NCORES)))
    part = sum(float(res.results[c]["out"][0, 0]) for c in range(NCORES))
    loss = -part / (T * B)
    return np.float32(loss)



# revision 2
# speedup vs baseline: 6.1479x; 6.1479x over previous
"""Trainium2 Bass kernel for the histogram-binning CTC-style loss.

reference:
    x: [T=512, B=32, V=10000] f32, label: [B=32, L=64] int
    counts[b,v] = histogram of non-blank labels; counts[b,0] = T - len_b
    loss = -sum_{b,v} (counts[b,v]/T) * log(mean_t x[t,b,v] + 1e-10) / B

Strategy (8 NeuronCores, data-parallel over batch):
    Only columns v with counts[b,v] != 0 contribute: <=64 unique labels +
    the blank per batch -> <=65 columns x 4 local batches per core out of
    40000.  The host ships x TRANSPOSED per core as xt[F=4*V, T] bf16 so
    each needed column is a 1KB contiguous row, plus a packed per-core
    table (int16 gather indices + f32 count weights).  The device runs TWO
    dma_gather instructions (SWDGE row gather; 256 idxs each covering a
    batch PAIR, keeping row ids < 2V = 20000 within int16 range), landing
    columns as xg[slot%128, pair*2 + slot//128, :].  T-sums are VectorE
    free-axis reduces, then ScalarE Ln(scale=1/T, bias=1e-10), a fused
    VectorE multiply+per-partition-sum against the weights, a PE
    ones-matmul partition reduce, and a 4B DMA out.  The host sums the 8
    per-core partials.

    One program serves any input: index tables and weights are data.
"""

import contextlib

import numpy as np
import ml_dtypes

import concourse.bacc as bacc
import concourse.mybir as mybir
import concourse.tile as tile
from concourse.bass_utils import run_bass_kernel_spmd

T = 512
B = 32
V = 10000
L = 64
NCORES = 8
BL = B // NCORES          # local batches per core
F = BL * V                # rows of the transposed per-core x
NG = BL // 2              # gathers per core (one per batch pair)
NIDX = 256                # indices per gather (128 slots x 2 batches)
ICOL = NIDX // 16         # free-dim columns of one gather's index table
# packed idx+w input: [128, NG*ICOL] int16 tables then [128, BL] f32
# weights bitcast to int16 columns
TWCOL = NG * ICOL + 2 * BL

_NC_CACHE = {}


def _build_nc(loop=1, bufs=1, unroll=1):
    nc = bacc.Bacc()
    xt_d = nc.declare_dram_parameter("xt", [F, T], mybir.dt.bfloat16,
                                     isOutput=False)
    tw_d = nc.declare_dram_parameter("tw", [128, TWCOL], mybir.dt.int16,
                                     isOutput=False)
    out_d = nc.declare_dram_parameter("out", [1, 1], mybir.dt.float32,
                                      isOutput=True)

    assert loop % unroll == 0
    trip = loop // unroll

    def body(sp, psum, biasv, ones):
        twt = sp.tile([128, TWCOL], mybir.dt.int16)
        nc.sync.dma_start(out=twt[:], in_=tw_d[:, :])
        idxt = twt[:, :NG * ICOL]
        wt = twt[:, NG * ICOL:].bitcast(mybir.dt.float32)

        xg = sp.tile([128, BL, T], mybir.dt.bfloat16)
        S = sp.tile([128, BL], mybir.dt.float32)
        for g in range(NG):
            nc.gpsimd.dma_gather(
                xg[:, 2 * g:2 * g + 2, :],
                xt_d[g * 2 * V:(g + 1) * 2 * V, :],
                idxt[:, g * ICOL:(g + 1) * ICOL],
                NIDX, NIDX, T,
            )
            nc.vector.tensor_reduce(
                out=S[:, 2 * g:2 * g + 2],
                in_=xg[:, 2 * g:2 * g + 2, :],
                axis=mybir.AxisListType.X,
                op=mybir.AluOpType.add,
            )
        logv = sp.tile([128, BL], mybir.dt.float32)
        nc.scalar.activation(
            logv[:], S[:], mybir.ActivationFunctionType.Ln,
            bias=biasv[:], scale=1.0 / T,
        )
        prod = sp.tile([128, BL], mybir.dt.float32)
        red = sp.tile([128, 1], mybir.dt.float32)
        nc.vector.scalar_tensor_tensor(
            out=prod[:], in0=logv[:], scalar=1.0, in1=wt,
            op0=mybir.AluOpType.mult, op1=mybir.AluOpType.mult,
            accum_out=red[:],
        )
        ps = psum.tile([1, 1], mybir.dt.float32)
        nc.tensor.matmul(ps[:], ones[:], red[:], start=True, stop=True)
        total = sp.tile([1, 1], mybir.dt.float32)
        nc.scalar.copy(total[:], ps[:])
        nc.sync.dma_start(out=out_d[:, :], in_=total[:])

    with tile.TileContext(nc) as tc:
        with tc.tile_pool(name="const", bufs=1) as cp:
            biasv = cp.tile([128, 1], mybir.dt.float32)
            nc.vector.memset(biasv[:], 1e-10)
            ones = cp.tile([128, 1], mybir.dt.float32)
            nc.vector.memset(ones[:], 1.0)
            # warm the ACT Ln table outside the loop
            warm = cp.tile([1, 1], mybir.dt.float32)
            nc.vector.memset(warm[:], 1.0)
            nc.scalar.activation(
                warm[:], warm[:], mybir.ActivationFunctionType.Ln,
                bias=biasv[0:1, :], scale=1.0,
            )

            loop_cm = (tc.For_i(0, trip, 1) if trip > 1
                       else contextlib.nullcontext())
            with (
                loop_cm,
                tc.tile_pool(name="sp", bufs=bufs) as sp,
                tc.tile_pool(name="psum", bufs=bufs, space="PSUM") as psum,
            ):
                for _u in range(unroll):
                    body(sp, psum, biasv, ones)

    nc.finalize()
    return nc


def get_nc():
    if "nc" not in _NC_CACHE:
        _NC_CACHE["nc"] = _build_nc()
    return _NC_CACHE["nc"]


def plan_core(label_rows):
    """label_rows: [BL, L] labels for one core's batches.
    Returns the packed [128, TWCOL] int16 per-core table: NG*ICOL columns
    of gather indices (idx i of gather g at [i % 16, g*ICOL + i // 16],
    replicated x8 down the partitions), then [128, BL] f32 count weights
    (slot p of batch b -> partition p, free col b) bitcast to int16."""
    idx = np.zeros((128, NG * ICOL), dtype=np.int16)
    w = np.zeros((128, BL), dtype=np.float32)
    for g in range(NG):
        slots = np.zeros(NIDX, dtype=np.int16)  # rid within the pair block
        for c in range(2):
            b = 2 * g + c
            lab = np.asarray(label_rows[b])
            m = lab != 0
            vals, cnts = np.unique(lab[m], return_counts=True)
            assert len(vals) < 128
            base = c * 128
            slots[base] = c * V  # blank column of this batch
            w[0, b] = T - m.sum()
            slots[base + 1:base + 1 + len(vals)] = c * V + vals
            w[1:1 + len(vals), b] = cnts
            # pad slots keep rid = c*V (blank), weight 0
            slots[base + 1 + len(vals):base + 128] = c * V
        # table layout: idx i lives at [i % 16, i // 16], replicated x8
        block = slots.reshape(ICOL, 16).T  # [16, ICOL]
        idx[:, g * ICOL:(g + 1) * ICOL] = np.tile(block, (8, 1))
    tw = np.concatenate([idx, w.view(np.int16)], axis=1)
    return np.ascontiguousarray(tw)


def make_in_maps(x, label):
    label = np.asarray(label)
    # [T, B, V] -> [B, V, T] bf16 contiguous
    xt = np.asarray(x).transpose(1, 2, 0).astype(ml_dtypes.bfloat16)
    in_maps = []
    for c in range(NCORES):
        xc = xt[c * BL:(c + 1) * BL].reshape(F, T)
        tw = plan_core(label[c * BL:(c + 1) * BL])
        in_maps.append({"xt": np.ascontiguousarray(xc), "tw": tw})
    return in_maps


def kernel(x, label):
    nc = get_nc()
    in_maps = make_in_maps(x, label)
    res = run_bass_kernel_spmd(nc, in_maps, core_ids=list(range(NCORES)))
    part = sum(float(res.results[c]["out"][0, 0]) for c in range(NCORES))
    loss = -part / (T * B)
    return np.float32(loss)
